# revision 5
# baseline (speedup 1.0000x reference)
"""Trainium2 Bass kernel v10 for the CCS primal-dual iteration (dense_cnn).

Math per image (10 iterations), EPS=TAU=1, q state stored PRE-relu:
    u = sigmoid(os); q = 0
    repeat 10x:
        dyu = u[y+1]-u ; dxu = u[x+1]-u          (zero-padded fwd diffs)
        q   = relu(q) - dyu*vf1 - dxu*vf0
        p0 = relu(q)*vf1 ; p1 = relu(q)*vf0
        r  = os + (p0[y-1]-p0[y]) + (p1[x-1]-p1[x])
        u  = sigmoid(r)
    output = r

All f32 data (the iteration amplifies rounding noise ~1e4-1e5x; 16-bit
state/inputs fail the 2e-2 gate).  Matmuls run as float32r when FP32R
is set: 1 cycle/row at N=512 (4x cheaper than fp32 LOW_HIGH) -- verified
numerically equivalent for this kernel's +-1 stencil weights.

Engine assignment (measured costs):
  - GPSIMD unused (shares an SBUF port with DVE: concurrent gpsimd slows
    DVE ~2.8x -- strict net loss).
  - relu fused into DVE scalar_tensor_tensor (f32 stt == f32 tt cost).
  - PE: dyu (ident/nident) if DYU_ON_PE else DVE; dxu (fwd/bndf);
    r-assembly incl ident@os (os DMA-staged, never resident in SBUF).
  - act: psum->sbuf copies of dxu + sigmoid.

Layout per core: x-major; partition p holds x = 128*b + p, block b in
[0,8); free dim (b, y) with 2-col guards around each block's y-run.
"""

import sys

for _p in ("/opt/trn_rl_repo", "/root/.axon_site/_ro/trn_rl_repo"):
    if _p not in sys.path:
        sys.path.append(_p)

import numpy as np

import concourse.bass as bass
import concourse.bacc as bacc
import concourse.mybir as mybir
from concourse.tile import TileContext
from concourse import bass_utils

P = 128
NB = 8
H = 1024
YG = H + 4
GOF = 2
CH = 512
NCH = H // CH
NITER = 10
F32 = mybir.dt.float32
N_CORES = 8
FP32R = False
DYU_ON_PE = False

_CACHED = {}


def _build_lhs_pack() -> np.ndarray:
    I = np.eye(P, dtype=np.float32)
    nI = -I
    fwd = -I.copy()
    for m in range(P - 1):
        fwd[m + 1, m] = 1.0
    bndf = np.zeros((P, P), dtype=np.float32)
    bndf[0, P - 1] = 1.0
    bwd = -I.copy()
    for m in range(1, P):
        bwd[m - 1, m] = 1.0
    bndb = np.zeros((P, P), dtype=np.float32)
    bndb[P - 1, 0] = 1.0
    return np.concatenate([I, nI, fwd, bndf, bwd, bndb], axis=1)


def _emit_kernel(nc: bass.Bass):
    sub = mybir.AluOpType.subtract
    mult = mybir.AluOpType.mult
    amax = mybir.AluOpType.max
    sigmoid = mybir.ActivationFunctionType.Sigmoid
    copyf = mybir.ActivationFunctionType.Copy
    F32R = mybir.dt.float32r

    def mm(out, lhsT, rhs, **kw):
        if FP32R:
            nc.tensor.matmul(out, lhsT.bitcast(F32R), rhs.bitcast(F32R), **kw)
        else:
            nc.tensor.matmul(out, lhsT, rhs, **kw)

    os_d = nc.dram_tensor("osd", [P, NB * H], F32, kind="ExternalInput")
    u0_d = nc.dram_tensor("u0d", [P, NB * H], F32, kind="ExternalInput")
    vf0_d = nc.dram_tensor("vf0d", [P, NB * H], F32, kind="ExternalInput")
    vf1_d = nc.dram_tensor("vf1d", [P, NB * H], F32, kind="ExternalInput")
    lhs_d = nc.dram_tensor("lhsd", [P, 6 * P], F32, kind="ExternalInput")
    out_d = nc.dram_tensor("outd", [P, NB * H], F32, kind="ExternalOutput")

    with TileContext(nc) as tc:
        with (
            tc.tile_pool(name="big", bufs=1) as big,
            tc.tile_pool(name="t1p", bufs=3) as t1p,
            tc.tile_pool(name="t2p", bufs=3) as t2p,
            tc.tile_pool(name="tp1", bufs=3) as tp1,
            tc.tile_pool(name="osp", bufs=3) as osp,
            tc.tile_pool(name="psD", bufs=1, space="PSUM") as psD,
            tc.tile_pool(name="psA", bufs=3, space="PSUM") as psA,
            tc.tile_pool(name="psR", bufs=4, space="PSUM") as psR,
        ):
            vf0_sb = big.tile([P, NB * H], F32, tag="vf0")
            vf1_sb = big.tile([P, NB * H], F32, tag="vf1")
            q_sb = big.tile([P, NB * H], F32, tag="q")
            u_sb = big.tile([P, NB * YG], F32, tag="u")
            p0_sb = big.tile([P, NB * YG], F32, tag="p0")
            lhs_sb = big.tile([P, 6 * P], F32, tag="lhs")

            ident = lhs_sb[:, 0 * P:1 * P]
            nident = lhs_sb[:, 1 * P:2 * P]
            fwd = lhs_sb[:, 2 * P:3 * P]
            bndf = lhs_sb[:, 3 * P:4 * P]
            bwd = lhs_sb[:, 4 * P:5 * P]
            bndb = lhs_sb[:, 5 * P:6 * P]

            def cb(b):
                return slice(b * H, (b + 1) * H)

            def cc(b, yh):
                s = b * H + yh * CH
                return slice(s, s + CH)

            def gb(b, off=0):
                s = b * YG + GOF + off
                return slice(s, s + H)

            def gc(b, yh, off=0):
                s = b * YG + GOF + yh * CH + off
                return slice(s, s + CH)

            nc.sync.dma_start(out=lhs_sb[:], in_=lhs_d[:])
            nc.vector.memset(u_sb[:], 0.0)
            nc.vector.memset(p0_sb[:], 0.0)

            for b in range(NB):
                nc.sync.dma_start(out=vf1_sb[:, cb(b)], in_=vf1_d[:, cb(b)])
                nc.sync.dma_start(out=vf0_sb[:, cb(b)], in_=vf0_d[:, cb(b)])
            for b in range(NB):
                nc.sync.dma_start(out=u_sb[:, gb(b)], in_=u0_d[:, cb(b)])

            def emit_diffs(b):
                """PE groups for block b: dyu (if DYU_ON_PE) and dxu."""
                dy, dx = [], []
                for yh in range(NCH):
                    if DYU_ON_PE:
                        d_ps = psD.tile([P, CH], F32, tag="dps")
                        mm(d_ps[:], ident, u_sb[:, gc(b, yh, 1)],
                           start=True, stop=False)
                        mm(d_ps[:], nident, u_sb[:, gc(b, yh)],
                           start=False, stop=True)
                        dy.append(d_ps)
                    a_ps = psA.tile([P, CH], F32, tag="aps")
                    mm(a_ps[:], fwd, u_sb[:, gc(b, yh)],
                       start=True, stop=True)
                    dx.append(a_ps)
                return dy, dx

            p1_prev = None
            diffs_next = emit_diffs(0)
            for it in range(NITER):
                last = it == NITER - 1
                for b in range(NB):
                    dyu_ps, dxu_ps = diffs_next
                    if b < NB - 1:
                        diffs_next = emit_diffs(b + 1)
                    elif it < NITER - 1:
                        diffs_next = emit_diffs(0)

                    g = gb(b)
                    c = cb(b)
                    # --- act: dxs = copy(dxu_ps) psum->sbuf ---
                    t2 = t2p.tile([P, H], F32, tag="t2")
                    for yh in range(NCH):
                        nc.scalar.activation(t2[:, yh * CH:(yh + 1) * CH],
                                             dxu_ps[yh][:], copyf)
                    if b < NB - 1:
                        # dxu[127,:] += u[0, b+1, :]  (cross-block fwd-diff term)
                        nc.gpsimd.dma_start(out=t2[127:128, :],
                                            in_=u_sb[0:1, gb(b + 1)],
                                            accum_op=mybir.AluOpType.add)
                    t1 = t1p.tile([P, H], F32, tag="t1")
                    if DYU_ON_PE:
                        # --- DVE: m1 = dyu_ps * vf1 (psum src) ---
                        for yh in range(NCH):
                            nc.vector.tensor_tensor(
                                out=t1[:, yh * CH:(yh + 1) * CH],
                                in0=dyu_ps[yh][:], in1=vf1_sb[:, cc(b, yh)],
                                op=mult)
                    else:
                        nc.vector.tensor_tensor(out=t1[:],
                                                in0=u_sb[:, gb(b, 1)],
                                                in1=u_sb[:, g], op=sub)
                        nc.vector.tensor_tensor(out=t1[:], in0=t1[:],
                                                in1=vf1_sb[:, c], op=mult)
                    # --- DVE: m2 = dxs * vf0 ---
                    nc.vector.tensor_tensor(out=t2[:], in0=t2[:],
                                            in1=vf0_sb[:, c], op=mult)
                    if it == 0:
                        # q starts at 0: q = -m1 - m2 in one stt
                        nc.vector.scalar_tensor_tensor(
                            out=q_sb[:, c], in0=t1[:], scalar=-1.0,
                            in1=t2[:], op0=mult, op1=sub)
                    else:
                        # --- DVE: a = relu(q) - m1 (stt) ---
                        nc.vector.scalar_tensor_tensor(
                            out=t1[:], in0=q_sb[:, c], scalar=0.0, in1=t1[:],
                            op0=amax, op1=sub)
                        # --- DVE: q = a - m2 (new pre-relu q) ---
                        nc.vector.tensor_tensor(out=q_sb[:, c], in0=t1[:],
                                                in1=t2[:], op=sub)
                    # --- DVE: p0 = relu(q)*vf1 (stt) ---
                    nc.vector.scalar_tensor_tensor(
                        out=p0_sb[:, g], in0=q_sb[:, c], scalar=0.0,
                        in1=vf1_sb[:, c], op0=amax, op1=mult)
                    # --- G2: p0 += c via accumulating DMA (CCE add) ---
                    nc.gpsimd.dma_start(out=p0_sb[:, g], in_=os_d[:, c],
                                        accum_op=mybir.AluOpType.add)
                    # --- DVE: p1 = relu(q)*vf0 (stt) ---
                    p1_t = tp1.tile([P, H], F32, tag="p1")
                    nc.vector.scalar_tensor_tensor(
                        out=p1_t[:], in0=q_sb[:, c], scalar=0.0,
                        in1=vf0_sb[:, c], op0=amax, op1=mult)

                    # --- PE: r = os + (p0[y-1]-p0[y]) + (p1[x-1]-p1[x]) ---
                    for yh in range(NCH):
                        cy = cc(b, yh)
                        r_ps = psR.tile([P, CH], F32, tag="rps")
                        mm(r_ps[:], ident, p0_sb[:, gc(b, yh, -1)],
                           start=True, stop=False)
                        mm(r_ps[:], nident, p0_sb[:, gc(b, yh)],
                           start=False, stop=False)
                        mm(r_ps[:], bwd, p1_t[:, yh * CH:(yh + 1) * CH],
                           start=False, stop=(b == 0))
                        if b > 0:
                            mm(r_ps[:], bndb,
                               p1_prev[:, yh * CH:(yh + 1) * CH],
                               start=False, stop=True)
                        uc = u_sb[:, gc(b, yh)]
                        if not last:
                            nc.scalar.activation(uc, r_ps[:], sigmoid)
                        else:
                            nc.scalar.activation(uc, r_ps[:], copyf)
                            nc.sync.dma_start(out=out_d[:, cy], in_=uc)
                    p1_prev = p1_t
    return nc


def _get_built():
    if "nc" not in _CACHED:
        nc = bacc.Bacc("TRN2")
        _emit_kernel(nc)
        nc.compile()
        _CACHED["nc"] = nc
        _CACHED["lhs"] = _build_lhs_pack()
    return _CACHED["nc"], _CACHED["lhs"]


def _to_core_layout(img: np.ndarray) -> np.ndarray:
    t = np.ascontiguousarray(img.T.astype(np.float32))
    t = t.reshape(NB, P, H).transpose(1, 0, 2)
    return np.ascontiguousarray(t.reshape(P, NB * H))


def _from_core_layout(flat: np.ndarray) -> np.ndarray:
    t = flat.astype(np.float32).reshape(P, NB, H).transpose(1, 0, 2)
    return np.ascontiguousarray(t.reshape(NB * P, H).T)


def kernel(o: np.ndarray, vector_field: np.ndarray, _trace=False):
    assert o.shape == (8, 1, 1024, 1024) and vector_field.shape == (8, 1024, 2, 1024)
    nc, lhs = _get_built()
    in_maps = []
    for ci in range(N_CORES):
        img = np.asarray(o[ci, 0], dtype=np.float32)
        u0 = (1.0 / (1.0 + np.exp(-img))).astype(np.float32)
        in_maps.append({
            "osd": _to_core_layout(-np.cumsum(img, axis=0, dtype=np.float32)),
            "u0d": _to_core_layout(u0),
            "vf0d": _to_core_layout(np.asarray(vector_field[ci, :, 0, :], np.float32)),
            "vf1d": _to_core_layout(np.asarray(vector_field[ci, :, 1, :], np.float32)),
            "lhsd": lhs,
        })

    res = bass_utils.run_bass_kernel_spmd(nc, in_maps, list(range(N_CORES)),
                                          trace=_trace)
    out = np.stack([_from_core_layout(res.results[ci]["outd"])
                    for ci in range(N_CORES)]).astype(np.float32)
    if _trace:
        return out, res
    return out
